# revision 2
# baseline (speedup 1.0000x reference)
"""GCN encoder (VGAE-style, 6 GCNConv) on 8 trn2 NeuronCores.

Strategy: partition nodes (and their aggregation work) across the 8 cores;
weights replicated. Per layer: each core computes table rows
h'[n] = dis[n] * (act[n] @ W) for its own 6250 nodes, an AllGather assembles
the full 50000x128 table on every core, then each core aggregates its own
edges (sorted by dst, grouped into 128-dst-node groups) with dma_gather +
one-hot indicator matmuls accumulating segment sums in PSUM.
norm factorization: norm[e] = dis[src]*dis[dst] is applied as a row scale on
the table (src side) and a per-partition scale at PSUM evacuation (dst side).
"""
import numpy as np

import concourse.bass as bass
import concourse.mybir as mybir
import concourse.tile as tile
import concourse.bacc as bacc
from concourse.bass_utils import run_bass_kernel_spmd

P = 128
NCORES = 8
N = 50000
E = 1600000
D = 128           # IN_C == HID == 128
OUTC = 64
NPC = N // NCORES          # 6250 nodes per core
G = (NPC + P - 1) // P     # 49 groups per core
LAST_ROWS = NPC - (G - 1) * P  # 106
SPLIT = 32768              # int16 index limit for dma_gather


def _set_dims(n, e):
    """Test hook: shrink the problem (n must be divisible by NCORES)."""
    global N, E, NPC, G, LAST_ROWS
    N, E = n, e
    NPC = N // NCORES
    G = (NPC + P - 1) // P
    LAST_ROWS = NPC - (G - 1) * P
SELU_L = 1.0507009873554805
SELU_A = 1.6732632423543772

f32 = mybir.dt.float32
i16 = mybir.dt.int16

_CACHE = {}


# ----------------------------------------------------------------- host prep
def _pack_idx16(vals, ntiles):
    """flat edge order i -> [128, ntiles*8] int16; i at (i%16, i//16), the
    16-row block replicated to all 8 gpsimd cores."""
    n = ntiles * P
    blk = np.full((16, n // 16), -1, np.int16)
    if len(vals):
        i = np.arange(len(vals))
        blk[i % 16, i // 16] = vals.astype(np.int16)
    return np.tile(blk, (8, 1))


def _preprocess(edge_index):
    src = np.asarray(edge_index[0], dtype=np.int64)
    dst = np.asarray(edge_index[1], dtype=np.int64)
    # self loops
    loops = np.arange(N, dtype=np.int64)
    src = np.concatenate([src, loops])
    dst = np.concatenate([dst, loops])

    deg = np.bincount(dst, minlength=N).astype(np.float32)  # includes self loop
    dis = 1.0 / np.sqrt(deg)

    core = dst // NPC
    g = (dst - core * NPC) // P
    gid = core * G + g
    segB = (src >= SPLIT).astype(np.int64)
    key = gid * 2 + segB
    order = np.lexsort((src, key))
    src_s, dst_s, key_s = src[order], dst[order], key[order]
    counts = np.bincount(key, minlength=NCORES * G * 2)
    starts = np.concatenate([[0], np.cumsum(counts)[:-1]])

    nA = counts[0::2].reshape(NCORES, G)
    nB = counts[1::2].reshape(NCORES, G)
    TA = int(np.ceil(nA.max() / P))
    TB = int(np.ceil(nB.max() / P))
    mnA = int(nA.min())
    mnB = int(nB.min())
    TT = TA + TB

    CH = 8  # tiles per dma_gather call; must match _build
    nchA = -(-TA // CH)
    nchB = -(-TB // CH) if TB else 0
    per_core = []
    for k in range(NCORES):
        idxA = np.zeros((P, G * TA * 8), np.int16)
        idxB = np.zeros((P, G * TB * 8), np.int16)
        dstloc = np.full((P, G * TT), 500.0, np.float32)
        gcnt = np.zeros((1, G * (nchA + nchB)), np.int32)
        for gg in range(G):
            base = k * NPC + gg * P
            for seg, (T, idxarr, coloff, nch, choff) in enumerate(
                    [(TA, idxA, 0, nchA, 0), (TB, idxB, TA, nchB, nchA)]):
                if T == 0:
                    continue
                kk = (k * G + gg) * 2 + seg
                s0, cnt = starts[kk], counts[kk]
                sv = src_s[s0:s0 + cnt] - (SPLIT if seg else 0)
                dv = dst_s[s0:s0 + cnt] - base
                flat = np.full(T * P, -1, np.int64)
                flat[:cnt] = sv
                # per-chunk valid counts (>=1; chunk-leading pad forced valid)
                for c in range(nch):
                    c0, c1 = c * CH * P, min((c + 1) * CH, T) * P
                    v = int(min(max(cnt - c0, 1), c1 - c0))
                    if cnt <= c0:
                        flat[c0] = 0
                    gcnt[0, gg * (nchA + nchB) + choff + c] = v
                idxarr[:, gg * T * 8:(gg + 1) * T * 8] = _pack_idx16(flat, T)
                i = np.arange(cnt)
                dstloc[i % P, gg * TT + coloff + i // P] = dv
        dis_k = np.zeros((P, G), np.float32)
        dcol = dis[k * NPC:(k + 1) * NPC]
        dis_k.T.flat[:NPC] = dcol          # dis_k[p, g] = dis[k*NPC + g*128 + p]
        per_core.append(dict(idxA=idxA, idxB=idxB, dstloc=dstloc, dis=dis_k,
                             gcnt=gcnt))
    return TA, TB, per_core, mnA, mnB


# ------------------------------------------------------------ device program
def _build(TA, TB, use_bias, mnA=0, mnB=0, sim_safe=False, n_passes=5, skip_own=False, skip_ag=False, repeat=1, only='full'):
    TT = TA + TB
    nc = bacc.Bacc("TRN2", target_bir_lowering=False, debug=False,
                   enable_asserts=False, num_devices=NCORES)

    def inp(name, shape, dt=f32):
        return nc.dram_tensor(name, shape, dt, kind="ExternalInput")

    idxA_in = inp("idxA", [P, G * TA * 8], i16)
    idxB_in = inp("idxB", [P, G * TB * 8], i16) if TB else None
    dstloc_in = inp("dstloc", [P, G * TT])
    iota_in = inp("iota", [P, P])
    ident_in = inp("ident", [P, P])
    dis_in = inp("dis_sc", [P, G])
    dis_lam_in = inp("dis_lam", [P, G])
    dis_neg_in = inp("dis_neg", [P, G])
    xT_in = inp("xT", [P, G * P])
    CH = 8
    nchA = -(-TA // CH)
    nchB = -(-TB // CH) if TB else 0
    gcnt_in = inp("gcnt", [1, G * (nchA + nchB)], mybir.dt.int32)
    w_in = [inp(f"W{i}", [P, P]) for i in range(5)]  # W0..W3, Wmulv
    bb_in = [inp(f"BB{i}", [P, P]) for i in range(5)] if use_bias else []

    mu_out = nc.dram_tensor("mu_out", [NPC, OUTC], f32, kind="ExternalOutput")
    lv_out = nc.dram_tensor("lv_out", [NPC, OUTC], f32, kind="ExternalOutput")

    h_own = nc.dram_tensor("h_own", [NPC, D], f32)
    tables = [nc.dram_tensor(f"table{i}", [N, D], f32, addr_space="Shared")
              for i in range(2)]

    RG = [list(range(NCORES))]
    AF = mybir.ActivationFunctionType

    with tile.TileContext(nc) as tc:
        with (
            tc.tile_pool(name="const", bufs=1) as cpool,
            tc.tile_pool(name="msg", bufs=3) as msg_pool,
            tc.tile_pool(name="ind", bufs=4) as ind_pool,
            tc.tile_pool(name="act", bufs=3) as act_pool,
            tc.tile_pool(name="tmp", bufs=4) as tmp_pool,
            tc.tile_pool(name="hps", bufs=3, space="PSUM") as agg_psum,
            tc.tile_pool(name="tps", bufs=2, space="PSUM") as tr_psum,
            tc.tile_pool(name="zps", bufs=2, space="PSUM") as z_psum,
        ):
            def load(ap_in, shape, tag, dt=f32):
                t = cpool.tile(shape, dt, tag=tag)
                nc.sync.dma_start(out=t[:], in_=ap_in[:, :])
                return t

            idxA = load(idxA_in, [P, G * TA * 8], "c_idxA", i16)
            idxB = (load(idxB_in, [P, G * TB * 8], "c_idxB", i16)
                    if TB else None)
            dstloc = load(dstloc_in, [P, G * TT], "c_dstloc")
            iota = load(iota_in, [P, P], "c_iota")
            ident = load(ident_in, [P, P], "c_ident")
            dis_sc = load(dis_in, [P, G], "c_dis")
            dis_lam = load(dis_lam_in, [P, G], "c_dlam")
            dis_neg = load(dis_neg_in, [P, G], "c_dneg")
            xT = load(xT_in, [P, G * P], "c_xT")
            gcnt = load(gcnt_in, [1, G * (nchA + nchB)], "c_gcnt",
                        mybir.dt.int32)
            cnt_regs = [nc.gpsimd.alloc_register(f"cntreg{i}")
                        for i in range(4)]
            cnt_rr = [0]

            def load_cnt(col, maxv):
                r = cnt_regs[cnt_rr[0] % 4]
                cnt_rr[0] += 1
                nc.gpsimd.reg_load(r, gcnt[0:1, col:col + 1])
                return r
            W = [load(w, [P, P], f"c_W{i}") for i, w in enumerate(w_in)]
            BB = ([load(b, [P, P], f"c_BB{i}") for i, b in enumerate(bb_in)]
                  if use_bias else None)

            def own_rows(g, z_ps, h_dram):
                """scale z (PSUM [128 nodes, D]) by dis and store node rows."""
                rows = P if g < G - 1 else LAST_ROWS
                h = act_pool.tile([P, D], f32, tag="hrow")
                nc.scalar.mul(h[:], z_ps[:], dis_sc[:, g:g + 1])
                nc.sync.dma_start(out=h_dram[g * P:g * P + rows, :],
                                  in_=h[:rows, :])

            # ---- prologue: table0 rows = dis * (x @ W0)
            for _rep in range(repeat):
             for g in range(G):
                z = z_psum.tile([P, D], f32, space="PSUM")
                nc.tensor.matmul(out=z[:], lhsT=xT[:, g * P:(g + 1) * P],
                                 rhs=W[0][:], start=True, stop=True)
                own_rows(g, z, h_own)
             if not skip_ag:
                nc.gpsimd.collective_compute(
                    "AllGather", mybir.AluOpType.bypass, replica_groups=RG,
                    ins=[h_own.ap().opt()], outs=[tables[0].ap().opt()])

             # ---- 5 aggregation passes
             passes = [("selu", 1), ("silu", 2), ("silu", 3),
                       ("softplus_neg", 4), ("final", None)]
             passes = [("final", None)] if n_passes == -1 else passes[:n_passes]
             for pi, (fn, wnext) in enumerate(passes):
                tbl = tables[pi % 2]
                tbl_next = tables[(pi + 1) % 2]
                for g in range(G):
                    msg = msg_pool.tile([P, TT, D], f32)
                    gbase = g * (nchA + nchB)
                    for c in range(nchA):
                        c0, c1 = c * CH, min((c + 1) * CH, TA)
                        if c1 * P > mnA:  # chunk may contain pad slots
                            m0 = max(c0, mnA // P)
                            nc.vector.memset(msg[:, m0:c1, :], 0.0)
                        nv = load_cnt(gbase + c, (c1 - c0) * P)
                        nc.gpsimd.dma_gather(
                            msg[:, c0:c1, :], tbl[0:min(SPLIT, N), :],
                            idxA[:, (g * TA + c0) * 8:(g * TA + c1) * 8],
                            (c1 - c0) * P, nv, D,
                            single_packet=(c1 - c0) * P <= 1024)
                    for c in range(nchB):
                        c0, c1 = c * CH, min((c + 1) * CH, TB)
                        if c1 * P > mnB:
                            m0 = max(c0, mnB // P)
                            nc.vector.memset(msg[:, TA + m0:TA + c1, :], 0.0)
                        nv = load_cnt(gbase + nchA + c, (c1 - c0) * P)
                        nc.gpsimd.dma_gather(
                            msg[:, TA + c0:TA + c1, :], tbl[SPLIT:N, :],
                            idxB[:, (g * TB + c0) * 8:(g * TB + c1) * 8],
                            (c1 - c0) * P, nv, D,
                            single_packet=(c1 - c0) * P <= 1024)
                    if only in ('gather', 'gather_half'):
                        continue
                    ps = agg_psum.tile([P, D], f32, space="PSUM")
                    for t in range(TT):
                        if only != 'nodve':
                            ind = ind_pool.tile([P, P], f32)
                            nc.vector.tensor_scalar(
                                out=ind[:], in0=iota[:],
                                scalar1=dstloc[:, g * TT + t:g * TT + t + 1],
                                scalar2=None, op0=mybir.AluOpType.is_equal)
                        else:
                            ind = iota
                        if only != 'nomm':
                            nc.tensor.matmul(out=ps[:], lhsT=ind[:],
                                             rhs=msg[:, t, :],
                                             start=(t == 0), stop=(t == TT - 1))
                    if only == 'nomm':
                        continue
                    # ---- evacuation: act = f(dis * ps + b)
                    act = act_pool.tile([P, D], f32, tag="act")
                    if use_bias:
                        lin = tmp_pool.tile([P, D], f32, tag="lin")
                        nc.vector.tensor_scalar(
                            out=lin[:], in0=ps[:],
                            scalar1=dis_sc[:, g:g + 1], scalar2=None,
                            op0=mybir.AluOpType.mult)
                        nc.vector.tensor_tensor(
                            out=lin[:], in0=lin[:], in1=BB[pi][:],
                            op=mybir.AluOpType.add)
                        srcx, s_sil, s_lam, s_neg = lin, 1.0, SELU_L, -1.0
                    else:
                        srcx = ps
                        s_sil = dis_sc[:, g:g + 1]
                        s_lam = dis_lam[:, g:g + 1]
                        s_neg = dis_neg[:, g:g + 1]
                    if fn == "silu":
                        if sim_safe:
                            sg = tmp_pool.tile([P, D], f32, tag="sg")
                            xx = tmp_pool.tile([P, D], f32, tag="xx")
                            nc.scalar.activation(sg[:], srcx[:], AF.Sigmoid,
                                                 scale=s_sil)
                            nc.scalar.mul(xx[:], srcx[:], s_sil)
                            nc.vector.tensor_tensor(
                                out=act[:], in0=sg[:], in1=xx[:],
                                op=mybir.AluOpType.mult)
                        else:
                            nc.scalar.activation(act[:], srcx[:], AF.Silu,
                                                 scale=s_sil)
                    elif fn == "softplus_neg":
                        # softplus(-x) = ln(1 + exp(-x))
                        e = tmp_pool.tile([P, D], f32, tag="sp_e")
                        nc.scalar.activation(e[:], srcx[:], AF.Exp,
                                             scale=s_neg)
                        nc.scalar.activation(act[:], e[:], AF.Ln, bias=1.0)
                    elif fn == "selu":
                        r = tmp_pool.tile([P, D], f32, tag="selu_r")
                        m = tmp_pool.tile([P, D], f32, tag="selu_m")
                        nc.scalar.activation(r[:], srcx[:], AF.Relu,
                                             scale=s_lam)
                        nc.scalar.activation(m[:], srcx[:], AF.Relu,
                                             scale=s_neg)
                        nc.scalar.activation(m[:], m[:], AF.Exp, scale=-1.0)
                        nc.vector.tensor_scalar(
                            out=m[:], in0=m[:],
                            scalar1=SELU_L * SELU_A, scalar2=-SELU_L * SELU_A,
                            op0=mybir.AluOpType.mult, op1=mybir.AluOpType.add)
                        nc.vector.tensor_tensor(out=act[:], in0=r[:],
                                                in1=m[:],
                                                op=mybir.AluOpType.add)
                    else:  # final
                        if use_bias:
                            nc.vector.tensor_copy(act[:], srcx[:])
                        else:
                            nc.scalar.mul(act[:], ps[:], dis_sc[:, g:g + 1])

                    rows = P if g < G - 1 else LAST_ROWS
                    if wnext is None:
                        nc.sync.dma_start(out=mu_out[g * P:g * P + rows, :],
                                          in_=act[:rows, 0:OUTC])
                        nc.sync.dma_start(out=lv_out[g * P:g * P + rows, :],
                                          in_=act[:rows, OUTC:D])
                    elif skip_own:
                        pass
                    else:
                        # own-rows stage for the next table
                        pT = tr_psum.tile([P, P], f32, space="PSUM")
                        nc.tensor.transpose(out=pT[:], in_=act[:],
                                            identity=ident[:])
                        hsT = tmp_pool.tile([P, P], f32, tag="hsT")
                        nc.vector.tensor_copy(hsT[:], pT[:])
                        z = z_psum.tile([P, D], f32, space="PSUM")
                        nc.tensor.matmul(out=z[:], lhsT=hsT[:],
                                         rhs=W[wnext][:],
                                         start=True, stop=True)
                        own_rows(g, z, h_own)
                if wnext is not None and not skip_own and not skip_ag:
                    nc.gpsimd.collective_compute(
                        "AllGather", mybir.AluOpType.bypass, replica_groups=RG,
                        ins=[h_own.ap().opt()],
                        outs=[tbl_next.ap().opt()])
    nc.finalize()
    return nc


# ------------------------------------------------------------------- driver
def _make_in_maps(x, per_core, TA, TB, Ws, biases=None):
    iota = np.tile(np.arange(P, dtype=np.float32), (P, 1))
    ident = np.eye(P, dtype=np.float32)
    in_maps = []
    for k in range(NCORES):
        pc = per_core[k]
        dis_k = pc["dis"]
        xT = np.zeros((P, G * P), np.float32)
        xT[:, :NPC] = x[k * NPC:(k + 1) * NPC].T
        m = dict(idxA=pc["idxA"], dstloc=pc["dstloc"], gcnt=pc["gcnt"],
                 iota=iota, ident=ident, dis_sc=dis_k,
                 dis_lam=(SELU_L * dis_k).astype(np.float32),
                 dis_neg=(-dis_k).astype(np.float32), xT=xT)
        if TB:
            m["idxB"] = pc["idxB"]
        for i, w in enumerate(Ws):
            m[f"W{i}"] = w
        if biases is not None:
            # bias of pass pi is conv pi's bias, broadcast across partitions
            for i, b in enumerate(biases):
                bb = np.tile(np.asarray(b, dtype=np.float32)[None, :], (P, 1))
                m[f"BB{i}"] = bb.astype(np.float32)
        in_maps.append(m)
    return in_maps


def _bench_prep(inputs):
    TA, TB, per_core, mnA, mnB = _preprocess(inputs["edge_index"])
    Wmulv = np.concatenate([-np.asarray(inputs["Wmu"]),
                            -np.asarray(inputs["Wlv"])], axis=1).astype(np.float32)
    Ws = [np.asarray(inputs[k], dtype=np.float32)
          for k in ("W0", "W1", "W2", "W3")] + [Wmulv]
    in_maps = _make_in_maps(np.asarray(inputs["x"], np.float32), per_core,
                            TA, TB, Ws, None)
    return dict(TA=TA, TB=TB, mnA=mnA, mnB=mnB, in_maps=in_maps)


def _bench_build(prep, repeat=1):
    return _build(prep["TA"], prep["TB"], False, prep["mnA"], prep["mnB"],
                  repeat=repeat)


def kernel(x, edge_index, W0, b0, W1, b1, W2, b2, W3, b3, Wmu, bmu, Wlv, blv):
    x = np.asarray(x, dtype=np.float32)
    edge_index = np.asarray(edge_index)
    assert x.shape == (N, D) and edge_index.shape == (2, E)

    TA, TB, per_core, mnA, mnB = _preprocess(edge_index)
    use_bias = any(np.any(np.asarray(b)) for b in (b0, b1, b2, b3, bmu, blv))

    key = (TA, TB, use_bias)
    if key not in _CACHE:
        _CACHE[key] = _build(TA, TB, use_bias, mnA, mnB)
    nc = _CACHE[key]

    Wmulv = np.concatenate([-np.asarray(Wmu), -np.asarray(Wlv)],
                           axis=1).astype(np.float32)
    Ws = [np.asarray(w, dtype=np.float32) for w in (W0, W1, W2, W3)] + [Wmulv]
    biases = None
    if use_bias:
        bmulv = np.concatenate([np.asarray(bmu), np.asarray(blv)])
        biases = (b0, b1, b2, b3, bmulv)
    in_maps = _make_in_maps(x, per_core, TA, TB, Ws, biases)

    res = run_bass_kernel_spmd(nc, in_maps, core_ids=list(range(NCORES)))
    mu = np.concatenate([res.results[k]["mu_out"] for k in range(NCORES)], axis=0)
    lv = np.concatenate([res.results[k]["lv_out"] for k in range(NCORES)], axis=0)
    return (mu, lv)



# revision 3
# speedup vs baseline: 1.7785x; 1.7785x over previous
"""GCN encoder (VGAE-style, 6 GCNConv) on 8 trn2 NeuronCores — v2 (bf16).

Strategy: partition nodes across 8 cores; weights replicated. Per layer:
each core computes its own table rows h'[n] = dis[n] * (act[n] @ W), an
AllGather assembles the full 50000x128 bf16 table, then each core aggregates
its own edges (sorted by dst, grouped into 128-dst blocks) with dma_gather +
one-hot indicator matmuls accumulating in PSUM.

v2 vs v1: everything bf16 (tables/messages/weights/indicators — halves the
gather DMA, 4x faster PE); aggregation runs transposed
(psT[feat,dst] = sum_t matmul(lhsT=msg_t, rhs=ind_t)) so the next layer's
matmul consumes actT directly with no PE transpose; dst-side norm is folded
into the indicator (ind = (iota==dstloc)*dis_dst, one wide broadcast DVE op
pair per group-pair); gathers use full-tile counts with idx=0 padding (no
memsets, no runtime count registers); groups are paired per dma_gather call
(half the SWDGE fixed cost).

Tile numbering within a pair (also the dstloc/dismsg column order):
  [A-tiles of g0 | A-tiles of g1 | B-tiles of g0 | B-tiles of g1]
"""
import numpy as np
import ml_dtypes

import concourse.bass as bass
import concourse.mybir as mybir
import concourse.tile as tile
import concourse.bacc as bacc
from concourse.bass_utils import run_bass_kernel_spmd

P = 128
NCORES = 8
N = 50000
E = 1600000
D = 128           # IN_C == HID == 128
OUTC = 64
NPC = N // NCORES          # 6250 nodes per core
G = (NPC + P - 1) // P     # 49 groups per core
LAST_ROWS = NPC - (G - 1) * P  # 106
SPLIT = 32768              # int16 index limit for dma_gather
SELU_L = 1.0507009873554805
SELU_A = 1.6732632423543772

f32 = mybir.dt.float32
bf16 = mybir.dt.bfloat16
i16 = mybir.dt.int16
BF = ml_dtypes.bfloat16

_CACHE = {}


def _pairs():
    return [(2 * i, 2 * i + 1 if 2 * i + 1 < G else None)
            for i in range((G + 1) // 2)]


# ----------------------------------------------------------------- host prep
def _pack_idx16(vals, ntiles):
    """flat edge order i -> [128, ntiles*8] int16; i at (i%16, i//16), the
    16-row block replicated to all 8 gpsimd cores."""
    n = ntiles * P
    blk = np.zeros((16, n // 16), np.int16)
    if len(vals):
        i = np.arange(len(vals))
        blk[i % 16, i // 16] = vals.astype(np.int16)
    return np.tile(blk, (8, 1))


def _preprocess(edge_index):
    src = np.asarray(edge_index[0], dtype=np.int64)
    dst = np.asarray(edge_index[1], dtype=np.int64)
    loops = np.arange(N, dtype=np.int64)
    src = np.concatenate([src, loops])
    dst = np.concatenate([dst, loops])

    deg = np.bincount(dst, minlength=N).astype(np.float32)
    dis = 1.0 / np.sqrt(np.maximum(deg, 1.0))

    core = dst // NPC
    g = (dst - core * NPC) // P
    gid = core * G + g
    segB = (src >= SPLIT).astype(np.int64)
    key = gid * 2 + segB
    order = np.lexsort((src, key))
    src_s, dst_s = src[order], dst[order]
    counts = np.bincount(key, minlength=NCORES * G * 2)
    starts = np.concatenate([[0], np.cumsum(counts)[:-1]])

    nA = counts[0::2].reshape(NCORES, G)
    nB = counts[1::2].reshape(NCORES, G)
    TA = int(np.ceil(nA.max() / P))
    TB = int(np.ceil(nB.max() / P))
    TT = TA + TB

    per_core = []
    for k in range(NCORES):
        idxA = np.zeros((P, G * TA * 8), np.int16)
        idxB = np.zeros((P, G * TB * 8), np.int16)
        # dstloc/dismsg columns follow pair-tile order; one column per tile.
        dstloc = np.full((P, G * TT), 500.0, np.float32)
        dismsg = np.zeros((P, G * TT), np.float32)
        for g0, g1 in _pairs():
            glist = [g0] if g1 is None else [g0, g1]
            ng = len(glist)
            blk = g0 * TT  # column offset of this pair's block
            for i_g, gg in enumerate(glist):
                base = k * NPC + gg * P
                for seg, (T, idxarr, tile_off) in enumerate(
                        [(TA, idxA, i_g * TA),
                         (TB, idxB, ng * TA + i_g * TB)]):
                    if T == 0:
                        continue
                    kk = (k * G + gg) * 2 + seg
                    s0, cnt = starts[kk], counts[kk]
                    sv = src_s[s0:s0 + cnt] - (SPLIT if seg else 0)
                    dv = dst_s[s0:s0 + cnt] - base
                    flat = np.zeros(T * P, np.int64)  # pad -> row 0 (finite)
                    flat[:cnt] = sv
                    idxarr[:, gg * T * 8:(gg + 1) * T * 8] = \
                        _pack_idx16(flat, T)
                    i = np.arange(cnt)
                    col = blk + tile_off + i // P
                    dstloc[i % P, col] = dv
                    dismsg[i % P, col] = dis[dst_s[s0:s0 + cnt]]
        dis_k = np.zeros((P, G), np.float32)
        dcol = dis[k * NPC:(k + 1) * NPC]
        dis_k.T.flat[:NPC] = dcol      # dis_k[p, g] = dis[k*NPC + g*128 + p]
        per_core.append(dict(idxA=idxA, idxB=idxB, dstloc=dstloc,
                             dismsg=dismsg, dis=dis_k))
    return TA, TB, per_core


# ------------------------------------------------------------ device program
def _build(TA, TB, use_bias, repeat=1, skip_ag=False, mode="full",
           cc_chunks=1, nqueues=4, gsplit="pair"):
    TT = TA + TB
    # chunk boundary (in groups) for cc_chunks=2: after pair 12 (groups 0-25)
    GSPLIT = 26
    RSPLIT = GSPLIT * P  # 3328
    nc = bacc.Bacc("TRN2", target_bir_lowering=False, debug=False,
                   enable_asserts=False, num_devices=NCORES,
                   num_swdge_queues=nqueues)
    qrr = [0]

    def next_q():
        q = qrr[0] % nqueues
        qrr[0] += 1
        return q

    def inp(name, shape, dt=bf16):
        return nc.dram_tensor(name, shape, dt, kind="ExternalInput")

    idxA_in = inp("idxA", [P, G * TA * 8], i16)
    idxB_in = inp("idxB", [P, G * TB * 8], i16)
    dstloc_in = inp("dstloc", [P, G * TT])
    dismsg_in = inp("dismsg", [P, G * TT])
    iota_in = inp("iota", [P, P])
    dis_in = inp("dis_sc", [P, G], f32)
    xT_in = inp("xT", [P, G * P])
    w_in = [inp(f"W{i}", [P, P]) for i in range(5)]  # W0..W3, Wmulv
    bT_in = inp("bT", [P, 4], f32) if use_bias else None   # b0..b3 per-feat
    bF_in = inp("bF", [P, P], f32) if use_bias else None   # final bias bcast

    mu_out = nc.dram_tensor("mu_out", [NPC, OUTC], f32, kind="ExternalOutput")
    lv_out = nc.dram_tensor("lv_out", [NPC, OUTC], f32, kind="ExternalOutput")

    if cc_chunks == 2:
        h_own0 = nc.dram_tensor("h_own0", [RSPLIT, D], bf16)
        h_own1 = nc.dram_tensor("h_own1", [NPC - RSPLIT, D], bf16)
    else:
        h_own = nc.dram_tensor("h_own", [NPC, D], bf16)
    tables = [nc.dram_tensor(f"table{i}", [N, D], bf16, addr_space="Shared")
              for i in range(2)]

    RG = [list(range(NCORES))]
    AF = mybir.ActivationFunctionType

    with tile.TileContext(nc) as tc:
        with (
            tc.tile_pool(name="const", bufs=1) as cpool,
            tc.tile_pool(name="msg", bufs=3) as msg_pool,
            tc.tile_pool(name="ind", bufs=2) as ind_pool,
            tc.tile_pool(name="act", bufs=3) as act_pool,
            tc.tile_pool(name="tmp", bufs=4) as tmp_pool,
            tc.tile_pool(name="hps", bufs=2, space="PSUM") as agg_psum,
            tc.tile_pool(name="zps", bufs=2, space="PSUM") as z_psum,
        ):
            def load(ap_in, shape, tag, dt=bf16):
                t = cpool.tile(shape, dt, tag=tag)
                nc.sync.dma_start(out=t[:], in_=ap_in[:, :])
                return t

            idxA = load(idxA_in, [P, G * TA * 8], "c_idxA", i16)
            idxB = load(idxB_in, [P, G * TB * 8], "c_idxB", i16)
            dstloc = load(dstloc_in, [P, G * TT], "c_dstloc")
            dismsg = load(dismsg_in, [P, G * TT], "c_dismsg")
            iota = load(iota_in, [P, P], "c_iota")
            dis_sc = load(dis_in, [P, G], "c_dis", f32)
            xT = load(xT_in, [P, G * P], "c_xT")
            W = [load(w, [P, P], f"c_W{i}") for i, w in enumerate(w_in)]
            bT = load(bT_in, [P, 4], "c_bT", f32) if use_bias else None
            bF = load(bF_in, [P, P], "c_bF", f32) if use_bias else None

            def own_rows(g, z_ps):
                """scale z (PSUM [128 nodes, D]) by dis, store bf16 rows."""
                rows = P if g < G - 1 else LAST_ROWS
                h = act_pool.tile([P, D], bf16, tag="hrow")
                nc.scalar.mul(h[:], z_ps[:], dis_sc[:, g:g + 1])
                if cc_chunks == 2 and g >= GSPLIT:
                    r0 = g * P - RSPLIT
                    nc.sync.dma_start(out=h_own1[r0:r0 + rows, :],
                                      in_=h[:rows, :])
                else:
                    hd = h_own0 if cc_chunks == 2 else h_own
                    nc.sync.dma_start(out=hd[g * P:g * P + rows, :],
                                      in_=h[:rows, :])

            def emit_cc(tbl_out, chunk=None):
                if skip_ag:
                    return
                if chunk is None:
                    nc.gpsimd.collective_compute(
                        "AllGather", mybir.AluOpType.bypass,
                        replica_groups=RG,
                        ins=[h_own.ap().opt()], outs=[tbl_out.ap().opt()])
                    return
                t3 = tbl_out.ap().rearrange("(c n) d -> c n d", c=NCORES)
                if chunk == 0:
                    nc.gpsimd.collective_compute(
                        "AllGather", mybir.AluOpType.bypass,
                        replica_groups=RG,
                        ins=[h_own0.ap().opt()],
                        outs=[t3[:, 0:RSPLIT, :]])
                else:
                    nc.gpsimd.collective_compute(
                        "AllGather", mybir.AluOpType.bypass,
                        replica_groups=RG,
                        ins=[h_own1.ap().opt()],
                        outs=[t3[:, RSPLIT:NPC, :]])

            def emit_cc_maybe(tbl_out, where):
                """where: 'mid' (after chunk-0 rows done) or 'end'."""
                if skip_ag:
                    return
                if cc_chunks == 2:
                    emit_cc(tbl_out, 0 if where == "mid" else 1)
                elif where == "end":
                    emit_cc(tbl_out)

            # ---- prologue: table0 rows = dis * (x @ W0)
            for _rep in range(repeat):
             for g in range(G):
                z = z_psum.tile([P, D], f32, space="PSUM")
                nc.tensor.matmul(out=z[:], lhsT=xT[:, g * P:(g + 1) * P],
                                 rhs=W[0][:], start=True, stop=True)
                own_rows(g, z)
                if g == GSPLIT - 1:
                    emit_cc_maybe(tables[0], "mid")
             emit_cc_maybe(tables[0], "end")

             # ---- 5 aggregation passes
             passes = [("selu", 1), ("silu", 2), ("silu", 3),
                       ("softplus_neg", 4), ("final", None)]
             for pi, (fn, wnext) in enumerate(passes):
                tbl = tables[pi % 2]
                tbl_next = tables[(pi + 1) % 2]
                for pairidx, (g0, g1) in enumerate(_pairs()):
                    if mode == "empty":
                        continue
                    glist = [g0] if g1 is None else [g0, g1]
                    ng = len(glist)
                    blk = g0 * TT
                    msg_full = msg_pool.tile([P, 2 * TT, D], bf16,
                                             tag="msg")
                    msg = msg_full[:, 0:ng * TT, :]
                    if gsplit == "pair":
                        nc.gpsimd.dma_gather(
                            msg[:, 0:ng * TA, :], tbl[0:SPLIT, :],
                            idxA[:, g0 * TA * 8:(g0 + ng) * TA * 8],
                            ng * TA * P, ng * TA * P, D,
                            single_packet=False, queue_num=next_q())
                        nc.gpsimd.dma_gather(
                            msg[:, ng * TA:ng * TT, :], tbl[SPLIT:N, :],
                            idxB[:, g0 * TB * 8:(g0 + ng) * TB * 8],
                            ng * TB * P, ng * TB * P, D,
                            single_packet=False, queue_num=next_q())
                    else:
                        ch = 8 if gsplit == "chunk8" else 10 ** 6
                        for i_g, gg in enumerate(glist):
                            for c0 in range(0, TA, ch):
                                c1 = min(c0 + ch, TA)
                                nc.gpsimd.dma_gather(
                                    msg[:, i_g * TA + c0:i_g * TA + c1, :],
                                    tbl[0:SPLIT, :],
                                    idxA[:, (gg * TA + c0) * 8:
                                         (gg * TA + c1) * 8],
                                    (c1 - c0) * P, (c1 - c0) * P, D,
                                    single_packet=(c1 - c0) * P <= 1024,
                                    queue_num=next_q())
                            for c0 in range(0, TB, ch):
                                c1 = min(c0 + ch, TB)
                                nc.gpsimd.dma_gather(
                                    msg[:, ng * TA + i_g * TB + c0:
                                        ng * TA + i_g * TB + c1, :],
                                    tbl[SPLIT:N, :],
                                    idxB[:, (gg * TB + c0) * 8:
                                         (gg * TB + c1) * 8],
                                    (c1 - c0) * P, (c1 - c0) * P, D,
                                    single_packet=(c1 - c0) * P <= 1024,
                                    queue_num=next_q())
                    if mode == "gather":
                        continue
                    # wide indicator (one op pair per group-pair):
                    # ind[p, t, j] = (iota[p,j] == dstloc[p, blk+t])
                    #                * dismsg[p, blk+t]
                    nw = ng * TT
                    ind_full = ind_pool.tile([P, 2 * TT, P], bf16,
                                             tag="ind")
                    ind = ind_full[:, 0:nw, :]
                    iota_b = iota[:, :].unsqueeze(1).to_broadcast([P, nw, P])
                    dl = dstloc[:, blk:blk + nw].unsqueeze(2).to_broadcast(
                        [P, nw, P])
                    dm = dismsg[:, blk:blk + nw].unsqueeze(2).to_broadcast(
                        [P, nw, P])
                    nc.vector.tensor_tensor(out=ind[:, :, :], in0=iota_b,
                                            in1=dl,
                                            op=mybir.AluOpType.is_equal)
                    nc.vector.tensor_tensor(out=ind[:, :, :],
                                            in0=ind[:, :, :],
                                            in1=dm, op=mybir.AluOpType.mult)
                    if mode == "nomm":
                        continue
                    ps = [agg_psum.tile([P, P], f32, space="PSUM",
                                        name=f"ps{i}", tag=f"ps{i}")
                          for i in range(ng)]
                    for i_g, g in enumerate(glist):
                        # this group's tiles: its A block then its B block
                        tlist = ([i_g * TA + t for t in range(TA)]
                                 + [ng * TA + i_g * TB + t
                                    for t in range(TB)])
                        for j, t in enumerate(tlist):
                            first, last = j == 0, j == len(tlist) - 1
                            if fn == "final":
                                nc.tensor.matmul(
                                    out=ps[i_g][:], lhsT=ind[:, t, :],
                                    rhs=msg[:, t, :],
                                    start=first, stop=last)
                            else:
                                nc.tensor.matmul(
                                    out=ps[i_g][:], lhsT=msg[:, t, :],
                                    rhs=ind[:, t, :],
                                    start=first, stop=last)
                    for i_g, g in enumerate(glist):
                        psg = ps[i_g]
                        # ---- evacuation
                        if fn == "final":
                            act = act_pool.tile([P, D], f32, tag="actf")
                            if use_bias:
                                nc.vector.tensor_tensor(
                                    out=act[:], in0=psg[:], in1=bF[:],
                                    op=mybir.AluOpType.add)
                            else:
                                nc.vector.tensor_copy(act[:], psg[:])
                            rows = P if g < G - 1 else LAST_ROWS
                            nc.sync.dma_start(
                                out=mu_out[g * P:g * P + rows, :],
                                in_=act[:rows, 0:OUTC])
                            nc.sync.dma_start(
                                out=lv_out[g * P:g * P + rows, :],
                                in_=act[:rows, OUTC:D])
                            continue
                        src_ps = psg
                        if use_bias:
                            lin = tmp_pool.tile([P, D], f32, tag="lin")
                            nc.vector.tensor_scalar(
                                out=lin[:], in0=psg[:],
                                scalar1=bT[:, pi:pi + 1], scalar2=None,
                                op0=mybir.AluOpType.add)
                            src_ps = lin
                        act = act_pool.tile([P, D], bf16, tag="act")
                        if fn == "silu":
                            nc.scalar.activation(act[:], src_ps[:], AF.Silu)
                        elif fn == "softplus_neg":
                            e = tmp_pool.tile([P, D], f32, tag="sp_e")
                            nc.scalar.activation(e[:], src_ps[:], AF.Exp,
                                                 scale=-1.0)
                            nc.scalar.activation(act[:], e[:], AF.Ln,
                                                 bias=1.0)
                        elif fn == "selu":
                            r = tmp_pool.tile([P, D], f32, tag="selu_r")
                            m = tmp_pool.tile([P, D], f32, tag="selu_m")
                            nc.scalar.activation(r[:], src_ps[:], AF.Relu,
                                                 scale=SELU_L)
                            nc.scalar.activation(m[:], src_ps[:], AF.Relu,
                                                 scale=-1.0)
                            nc.scalar.activation(m[:], m[:], AF.Exp,
                                                 scale=-1.0)
                            nc.vector.tensor_scalar(
                                out=m[:], in0=m[:],
                                scalar1=SELU_L * SELU_A,
                                scalar2=-SELU_L * SELU_A,
                                op0=mybir.AluOpType.mult,
                                op1=mybir.AluOpType.add)
                            nc.vector.tensor_tensor(
                                out=act[:], in0=r[:], in1=m[:],
                                op=mybir.AluOpType.add)
                        # ---- own rows for next table: z = act @ Wnext
                        # act is [feat, node]-major: lhsT=act directly.
                        z = z_psum.tile([P, D], f32, space="PSUM")
                        nc.tensor.matmul(out=z[:], lhsT=act[:],
                                         rhs=W[wnext][:],
                                         start=True, stop=True)
                        own_rows(g, z)
                    if wnext is not None and pairidx == GSPLIT // 2 - 1:
                        emit_cc_maybe(tbl_next, "mid")
                if wnext is not None:
                    emit_cc_maybe(tbl_next, "end")
    nc.finalize()
    return nc


# ------------------------------------------------------------------- driver
def _make_in_maps(x, per_core, TA, TB, Ws, biases=None):
    iota = np.tile(np.arange(P, dtype=np.float32), (P, 1))
    in_maps = []
    for k in range(NCORES):
        pc = per_core[k]
        xT = np.zeros((P, G * P), np.float32)
        xT[:, :NPC] = x[k * NPC:(k + 1) * NPC].T
        m = dict(idxA=pc["idxA"], idxB=pc["idxB"],
                 dstloc=pc["dstloc"].astype(BF),
                 dismsg=pc["dismsg"].astype(BF),
                 iota=iota.astype(BF), dis_sc=pc["dis"],
                 xT=xT.astype(BF))
        for i, w in enumerate(Ws):
            m[f"W{i}"] = np.asarray(w, np.float32).astype(BF)
        if biases is not None:
            bT = np.stack([np.asarray(b, np.float32) for b in biases[:4]],
                          axis=1)  # [128, 4]
            m["bT"] = bT.astype(np.float32)
            m["bF"] = np.tile(np.asarray(biases[4], np.float32)[None, :],
                              (P, 1)).astype(np.float32)
        in_maps.append(m)
    return in_maps


def _prep_all(inputs):
    TA, TB, per_core = _preprocess(inputs["edge_index"])
    Wmulv = np.concatenate([-np.asarray(inputs["Wmu"]),
                            -np.asarray(inputs["Wlv"])], axis=1)
    Ws = [inputs["W0"], inputs["W1"], inputs["W2"], inputs["W3"], Wmulv]
    use_bias = any(np.any(np.asarray(inputs[b]))
                   for b in ("b0", "b1", "b2", "b3", "bmu", "blv"))
    biases = None
    if use_bias:
        # pass-3 stores -log_sigmoid rows and Wmulv is negated, so the final
        # linear is  A(-h)(-W) + b  with the ORIGINAL bias sign.
        bmulv = np.concatenate([np.asarray(inputs["bmu"]),
                                np.asarray(inputs["blv"])])
        biases = [inputs["b0"], inputs["b1"], inputs["b2"], inputs["b3"],
                  bmulv]
    in_maps = _make_in_maps(np.asarray(inputs["x"], np.float32), per_core,
                            TA, TB, Ws, biases)
    return dict(TA=TA, TB=TB, use_bias=use_bias, in_maps=in_maps)


def _bench_prep(inputs):
    return _prep_all(inputs)


def _bench_build(prep, repeat=1, **kw):
    return _build(prep["TA"], prep["TB"], prep["use_bias"], repeat=repeat,
                  **kw)


def kernel(x, edge_index, W0, b0, W1, b1, W2, b2, W3, b3, Wmu, bmu, Wlv, blv):
    inputs = dict(x=x, edge_index=edge_index, W0=W0, b0=b0, W1=W1, b1=b1,
                  W2=W2, b2=b2, W3=W3, b3=b3, Wmu=Wmu, bmu=bmu, Wlv=Wlv,
                  blv=blv)
    x = np.asarray(x, dtype=np.float32)
    assert x.shape == (N, D) and np.asarray(edge_index).shape == (2, E)
    prep = _prep_all(inputs)

    key = (prep["TA"], prep["TB"], prep["use_bias"])
    if key not in _CACHE:
        _CACHE[key] = _build(*key)
    nc = _CACHE[key]

    res = run_bass_kernel_spmd(nc, prep["in_maps"],
                               core_ids=list(range(NCORES)))
    mu = np.concatenate([res.results[k]["mu_out"] for k in range(NCORES)],
                        axis=0)
    lv = np.concatenate([res.results[k]["lv_out"] for k in range(NCORES)],
                        axis=0)
    return (mu, lv)
